# revision 1
# baseline (speedup 1.0000x reference)
"""Gemma4 text attention as a TRN2 Bass kernel, tensor-parallel over 8 NeuronCores.

Problem (hardcoded): B=2, S=2048, D=2048, H=16 q-heads, KV=4 kv-heads, HD=256.

Sharding: core c owns q-heads {2c, 2c+1} and kv-head c//2 (Wq/Wk/Wv split
column-wise, Wo row-wise).  Each core computes a partial Y_c = attn_c @ Wo_c
over its 512 features; a per-batch ReduceScatter sums the partials on device
and leaves each core with a 256-token slice per batch, so only 4 MB/core goes
back to the host.  X^T arrives token-sharded (4 MB/core) and is AllGathered
on device.

Device pipeline per core (all matmuls in float32r = full-rate fp32):
  phase 1: X^T tiles -> Q^T/K^T (features on partitions) + V^T -> PE-transpose
           to V natural; per-head RMSNorm via gpsimd partition_all_reduce,
           (1+w) fold, RoPE on DVE.
  phase 2: per (batch, head): causal S^T = K^T.T@Q^T -> exp on ACT (f32r) ->
           band-mask on diagonal tiles -> PV and denominator (ones-vector)
           matmuls; 1/denominator via partition_broadcast folded into the
           PSUM->SBUF copy of O^T.
  phase 3: Y = O^T.T @ Wo per 128-token x 512-feature tile -> internal DRAM,
           then ReduceScatter(add) over the 8 cores.
"""

import numpy as np

import concourse.bass as bass
import concourse.mybir as mybir
import concourse.tile as tile
from concourse import bacc
from concourse.bass_isa import ReduceOp

# problem constants
B, S, D = 2, 2048, 2048
H, KV, HD = 16, 4, 256
ROPE_THETA = 10000.0
EPS = 1e-6
NCORES = 8
HPC = H // NCORES  # q heads per core = 2
EC = HPC * HD  # per-core feature width = 512
T = B * S  # total tokens = 4096
SH = S // NCORES  # token-shard width = 256

F32 = mybir.dt.float32
F32R = mybir.dt.float32r
BF16 = mybir.dt.bfloat16

TT = 512  # token tile (free dim) for projections
NTT = S // TT  # 4 token tiles per batch
DK = D // 128  # 16 contraction chunks
QC = EC // 128  # 4 q-feature chunks per core
KC = HD // 128  # 2 k-feature chunks per core
VS = S // 128  # 16 token subtiles per batch

_CACHE = {}


def _phase1(nc, tc, b, qt, kt, vt, xg, wq, wk, wv, cos_t, sin_t, w1pq, w1pk, eps_c):
    mult = mybir.AluOpType.mult
    with (
        tc.tile_pool(name=f"w{b}", bufs=1) as wpool,
        tc.tile_pool(name=f"p1t{b}", bufs=1) as p1t,
        tc.tile_pool(name=f"xt{b}", bufs=3) as xtp,
        tc.tile_pool(name=f"ps1_{b}", bufs=1, space="PSUM") as ps1,
    ):
        wq_t = wpool.tile([128, DK, EC], F32R)
        wk_t = wpool.tile([128, DK, HD], F32R)
        wv_t = wpool.tile([128, DK, HD], F32R)
        nc.sync.dma_start(wq_t[:], wq.rearrange("(ko ki) e -> ki ko e", ki=128).bitcast(F32R))
        nc.sync.dma_start(wk_t[:], wk.rearrange("(ko ki) e -> ki ko e", ki=128).bitcast(F32R))
        nc.sync.dma_start(wv_t[:], wv.rearrange("(ko ki) e -> ki ko e", ki=128).bitcast(F32R))
        ident = p1t.tile([128, 128], F32, tag="ident")
        nc.gpsimd.memset(ident[:], 1.0)
        nc.gpsimd.affine_select(
            out=ident[:], in_=ident[:], compare_op=mybir.AluOpType.is_equal,
            fill=0.0, base=0, channel_multiplier=1, pattern=[[-1, 128]],
        )
        for jt in range(NTT):
            psq = [ps1.tile([128, TT], F32, tag=f"psq{c}", name=f"psq{c}") for c in range(QC)]
            psk = [ps1.tile([128, TT], F32, tag=f"psk{c}", name=f"psk{c}") for c in range(KC)]
            psv = [ps1.tile([128, TT], F32, tag=f"psv{c}", name=f"psv{c}") for c in range(KC)]
            for d in range(DK):
                xt = xtp.tile([128, TT], F32R, tag="xt")
                # token tile jt spans gathered shards 2*jt and 2*jt+1
                for half in range(2):
                    sh = 2 * jt + half
                    eng = nc.sync if half == 0 else nc.gpsimd
                    eng.dma_start(
                        xt[:, half * SH : (half + 1) * SH],
                        xg[D * sh + d * 128 : D * sh + (d + 1) * 128, :].bitcast(F32R),
                    )
                st, sp = d == 0, d == DK - 1
                for c in range(QC):
                    nc.tensor.matmul(
                        psq[c][:], wq_t[:, d, c * 128 : (c + 1) * 128], xt[:], start=st, stop=sp
                    )
                for c in range(KC):
                    nc.tensor.matmul(
                        psk[c][:], wk_t[:, d, c * 128 : (c + 1) * 128], xt[:], start=st, stop=sp
                    )
                for c in range(KC):
                    nc.tensor.matmul(
                        psv[c][:], wv_t[:, d, c * 128 : (c + 1) * 128], xt[:], start=st, stop=sp
                    )
            # V^T: PSUM -> SBUF, then PE-transpose into natural V layout
            for c in range(KC):
                vtt = p1t.tile([128, TT], F32, tag=f"vtt{c}", name=f"vtt{c}")
                nc.scalar.copy(vtt[:], psv[c][:])
                for s in range(4):
                    tp = ps1.tile([128, 128], F32, tag=f"psv{c}", name=f"tp{c}{s}")
                    nc.tensor.transpose(tp[:], vtt[:, s * 128 : (s + 1) * 128], ident[:])
                    nc.scalar.copy(vt[:, 4 * jt + s, c * 128 : (c + 1) * 128], tp[:])
            # raw copies for q chunks (frees PSUM banks quickly)
            raw = []
            for c in range(QC):
                rawc = p1t.tile([128, TT], F32R, tag=f"raw{c}", name=f"raw{c}")
                nc.scalar.copy(rawc[:], psq[c][:])
                raw.append(rawc)
            # per head: rstd, then norm * (1+w), then rope.  kv head first so
            # its PSUM banks (read directly, no raw copy) free earliest.
            for h in (2, 0, 1):  # 0,1 = q heads; 2 = kv head
                if h < 2:
                    src0, src1 = raw[2 * h][:], raw[2 * h + 1][:]
                    w1p = w1pq
                    o0 = qt[:, 2 * h, jt * TT : (jt + 1) * TT]
                    o1 = qt[:, 2 * h + 1, jt * TT : (jt + 1) * TT]
                else:
                    src0, src1 = psk[0][:], psk[1][:]
                    w1p = w1pk
                    o0 = kt[:, 0, jt * TT : (jt + 1) * TT]
                    o1 = kt[:, 1, jt * TT : (jt + 1) * TT]
                sq_a = p1t.tile([128, TT], BF16, tag="m1", name="sq_a")
                nc.scalar.square(sq_a[:], src0)
                sq_b = p1t.tile([128, TT], BF16, tag="m2", name="sq_b")
                nc.scalar.square(sq_b[:], src1)
                sqs = p1t.tile([128, TT], F32, tag="n0", name="sqs")
                nc.vector.tensor_add(sqs[:], sq_a[:], sq_b[:])
                ssqb = p1t.tile([128, TT], F32, tag="ssqb", name="ssqb")
                nc.gpsimd.partition_all_reduce(ssqb[:], sqs[:], channels=128, reduce_op=ReduceOp.add)
                sroot = p1t.tile([128, TT], F32, tag="n1", name="sroot")
                nc.scalar.activation(
                    sroot[:], ssqb[:], mybir.ActivationFunctionType.Sqrt,
                    bias=eps_c[:], scale=1.0 / HD,
                )
                rstd = p1t.tile([128, TT], F32, tag="ssqb", name="rstd")
                nc.vector.reciprocal_approx_fast(rstd[:], sroot[:])
                n0 = p1t.tile([128, TT], F32, tag="n0", name="n0")
                n1 = p1t.tile([128, TT], F32, tag="n1", name="n1")
                nc.vector.scalar_tensor_tensor(
                    n0[:], src0, w1p[:, 0:1], rstd[:], op0=mult, op1=mult
                )
                nc.vector.scalar_tensor_tensor(
                    n1[:], src1, w1p[:, 1:2], rstd[:], op0=mult, op1=mult
                )
                cs = cos_t[:, jt * TT : (jt + 1) * TT]
                sn = sin_t[:, jt * TT : (jt + 1) * TT]
                m1 = p1t.tile([128, TT], F32, tag="m1", name="m1")
                m2 = p1t.tile([128, TT], F32, tag="m2", name="m2")
                nc.vector.tensor_mul(m1[:], n0[:], cs)
                nc.vector.tensor_mul(m2[:], n1[:], sn)
                nc.vector.tensor_sub(o0, m1[:], m2[:])
                m3 = p1t.tile([128, TT], F32, tag="m1", name="m3")
                m4 = p1t.tile([128, TT], F32, tag="m2", name="m4")
                nc.vector.tensor_mul(m3[:], n1[:], cs)
                nc.vector.tensor_mul(m4[:], n0[:], sn)
                nc.vector.tensor_add(o1, m3[:], m4[:])


def _phase2(nc, tc, b, qt, kt, vt, ot, ones, band, wo, wop):
    wo_t = wop.tile([128, QC, D], F32R)
    nc.sync.dma_start(wo_t[:], wo.rearrange("(eo ei) d -> ei eo d", ei=128).bitcast(F32R))
    with (
        tc.tile_pool(name=f"p2t{b}", bufs=1) as p2t,
        tc.tile_pool(name=f"es{b}", bufs=6) as esp,
        tc.tile_pool(name=f"ps2_{b}", bufs=1, space="PSUM") as ps2,
    ):
        for h in range(HPC):
            for j in range(NTT):
                nk = 4 * j + 4
                pso = [ps2.tile([128, TT], F32, tag=f"pso{c}_{j % 2}", name=f"pso{c}") for c in range(2)]
                psden = ps2.tile([1, TT], F32, tag="psden")
                for i in range(nk):
                    pss = ps2.tile([128, TT], F32, tag=f"pss{i % 3}")
                    for c in range(KC):
                        nc.tensor.matmul(
                            pss[:],
                            kt[:, c, i * 128 : (i + 1) * 128],
                            qt[:, 2 * h + c, j * TT : (j + 1) * TT],
                            start=(c == 0),
                            stop=(c == KC - 1),
                        )
                    es = esp.tile([128, TT], F32R, tag="es")
                    nc.scalar.activation(
                        es[:], pss[:], mybir.ActivationFunctionType.Exp,
                        scale=float(HD) ** -0.5,
                    )
                    if i >= 4 * j:
                        off = 384 - (128 * i - 512 * j)
                        nc.vector.tensor_mul(es[:], es[:], band[:, off : off + TT])
                    st, sp = i == 0, i == nk - 1
                    nc.tensor.matmul(pso[0][:], vt[:, i, 0:128], es[:], start=st, stop=sp)
                    nc.tensor.matmul(pso[1][:], vt[:, i, 128:256], es[:], start=st, stop=sp)
                    nc.tensor.matmul(psden[:], ones[:], es[:], start=st, stop=sp)
                den = p2t.tile([1, TT], F32, tag="den")
                nc.vector.tensor_copy(den[:], psden[:])
                rec = p2t.tile([1, TT], F32, tag="rec")
                nc.vector.reciprocal_approx_fast(rec[:], den[:])
                rbc = p2t.tile([128, TT], F32, tag="rbc")
                nc.gpsimd.partition_broadcast(rbc[:], rec[:])
                for c in range(2):
                    nc.vector.tensor_mul(
                        ot[:, 2 * h + c, j * TT : (j + 1) * TT], pso[c][:], rbc[:]
                    )
    return wo_t


def _phase3(nc, tc, b, ot, wo_t, yf):
    with (
        tc.tile_pool(name=f"p3t{b}", bufs=4) as p3t,
        tc.tile_pool(name=f"ps3_{b}", bufs=4, space="PSUM") as ps3,
    ):
        for tk in range(VS):
            for dt_ in range(4):
                psy = ps3.tile([128, TT], F32, tag="psy")
                for e in range(QC):
                    nc.tensor.matmul(
                        psy[:],
                        ot[:, e, tk * 128 : (tk + 1) * 128],
                        wo_t[:, e, dt_ * TT : (dt_ + 1) * TT],
                        start=(e == 0),
                        stop=(e == QC - 1),
                    )
                ysb = p3t.tile([128, TT], F32, tag="ysb")
                if (tk + dt_) % 2 == 0:
                    nc.scalar.copy(ysb[:], psy[:])
                else:
                    nc.vector.tensor_copy(ysb[:], psy[:])
                eng = nc.sync if dt_ % 2 == 0 else nc.gpsimd
                eng.dma_start(
                    yf[tk * 128 : (tk + 1) * 128, dt_ * TT : (dt_ + 1) * TT], ysb[:]
                )


def _build():
    nc = bacc.Bacc("TRN2", debug=False, num_devices=NCORES)
    groups = [list(range(NCORES))]

    x0 = nc.dram_tensor("x0", [D, SH], F32, kind="ExternalInput").ap()
    x1 = nc.dram_tensor("x1", [D, SH], F32, kind="ExternalInput").ap()
    wq = nc.dram_tensor("wq", [D, EC], F32, kind="ExternalInput").ap()
    wk = nc.dram_tensor("wk", [D, HD], F32, kind="ExternalInput").ap()
    wv = nc.dram_tensor("wv", [D, HD], F32, kind="ExternalInput").ap()
    wo = nc.dram_tensor("wo", [EC, D], F32, kind="ExternalInput").ap()
    cos_d = nc.dram_tensor("cos_t", [128, S], F32, kind="ExternalInput").ap()
    sin_d = nc.dram_tensor("sin_t", [128, S], F32, kind="ExternalInput").ap()
    w1pq_d = nc.dram_tensor("w1pq", [128, 2], F32, kind="ExternalInput").ap()
    w1pk_d = nc.dram_tensor("w1pk", [128, 2], F32, kind="ExternalInput").ap()
    y = nc.dram_tensor("y", [2 * SH, D], F32, kind="ExternalOutput").ap()

    with tile.TileContext(nc) as tc:
        with (
            tc.tile_pool(name="dram", bufs=1, space="DRAM") as dram,
            tc.tile_pool(name="consts", bufs=1) as consts,
        ):
            # gather the token-sharded X^T on device
            xb = [dram.tile([D, SH], F32, name=f"xb{bb}") for bb in range(B)]
            xg = [dram.tile([NCORES * D, SH], F32, name=f"xg{bb}") for bb in range(B)]
            for bb, xin in enumerate((x0, x1)):
                nc.sync.dma_start(xb[bb][:], xin)
                nc.gpsimd.collective_compute(
                    "AllGather",
                    mybir.AluOpType.bypass,
                    replica_groups=groups,
                    ins=[xb[bb][:].opt()],
                    outs=[xg[bb][:].opt()],
                )
            yf = [dram.tile([S, D], F32, name=f"yf{bb}") for bb in range(B)]
            yrs = [dram.tile([SH, D], F32, name=f"yrs{bb}") for bb in range(B)]

            cos_t = consts.tile([128, S], F32)
            sin_t = consts.tile([128, S], F32)
            nc.sync.dma_start(cos_t[:], cos_d)
            nc.sync.dma_start(sin_t[:], sin_d)
            w1pq = consts.tile([128, 2], F32)
            w1pk = consts.tile([128, 2], F32)
            nc.sync.dma_start(w1pq[:], w1pq_d)
            nc.sync.dma_start(w1pk[:], w1pk_d)
            eps_c = consts.tile([128, 1], F32)
            nc.vector.memset(eps_c[:], EPS)
            ones_f = consts.tile([128, 1], F32)
            nc.vector.memset(ones_f[:], 1.0)
            ones = consts.tile([128, 1], F32R)
            nc.vector.tensor_copy(ones[:], ones_f[:])
            band = consts.tile([128, 896], BF16)
            nc.gpsimd.memset(band[:], 1.0)
            nc.gpsimd.affine_select(
                out=band[:],
                in_=band[:],
                compare_op=mybir.AluOpType.is_ge,
                fill=0.0,
                base=-384,
                channel_multiplier=-1,
                pattern=[[1, 896]],
            )

            for b in range(B):
                with tc.tile_pool(name=f"ot{b}", bufs=1) as otp:
                    ot = otp.tile([128, QC, S], F32R)  # O^T, softmax-normalized
                    with tc.tile_pool(name=f"qkv{b}", bufs=1) as qkv:
                        qt = qkv.tile([128, QC, S], F32R)
                        kt = qkv.tile([128, KC, S], F32R)
                        vt = qkv.tile([128, VS, HD], F32R)
                        _phase1(nc, tc, b, qt, kt, vt, xg[b], wq, wk, wv,
                                cos_t, sin_t, w1pq, w1pk, eps_c)
                        with tc.tile_pool(name=f"wo{b}", bufs=1) as wop:
                            wo_t = _phase2(nc, tc, b, qt, kt, vt, ot, ones, band, wo, wop)
                            _phase3(nc, tc, b, ot, wo_t, yf[b])
                # sum partials across cores; each core keeps its token slice
                nc.gpsimd.collective_compute(
                    "ReduceScatter",
                    mybir.AluOpType.add,
                    replica_groups=groups,
                    ins=[yf[b][:].opt()],
                    outs=[yrs[b][:].opt()],
                )
                nc.sync.dma_start(y[b * SH : (b + 1) * SH, :], yrs[b][:])

    nc.compile()
    return nc


def get_nc():
    if "nc" not in _CACHE:
        _CACHE["nc"] = _build()
    return _CACHE["nc"]


def _get_runner():
    if "runner" in _CACHE:
        return _CACHE["runner"]
    import jax
    from jax.sharding import Mesh, PartitionSpec
    from jax.experimental.shard_map import shard_map
    from concourse import bass2jax
    from concourse.bass2jax import _bass_exec_p, install_neuronx_cc_hook

    nc = get_nc()
    install_neuronx_cc_hook()
    partition_name = nc.partition_id_tensor.name if nc.partition_id_tensor else None
    in_names, out_names, out_avals, zero_shapes = [], [], [], []
    for alloc in nc.m.functions[0].allocations:
        if not isinstance(alloc, mybir.MemoryLocationSet):
            continue
        name = alloc.memorylocations[0].name
        if alloc.kind == "ExternalInput":
            if name != partition_name:
                in_names.append(name)
        elif alloc.kind == "ExternalOutput":
            out_names.append(name)
            shape = tuple(alloc.tensor_shape)
            dtype = mybir.dt.np(alloc.dtype)
            out_avals.append(jax.core.ShapedArray(shape, dtype))
            zero_shapes.append((shape, dtype))
    n_params = len(in_names)
    n_outs = len(out_names)
    in_names_all = in_names + out_names + ([partition_name] if partition_name else [])
    donate = tuple(range(n_params, n_params + n_outs))

    def _body(*args):
        operands = list(args)
        if partition_name is not None:
            operands.append(bass2jax.partition_id_tensor())
        outs = _bass_exec_p.bind(
            *operands,
            out_avals=tuple(out_avals),
            in_names=tuple(in_names_all),
            out_names=tuple(out_names),
            lowering_input_output_aliases=(),
            sim_require_finite=True,
            sim_require_nnan=True,
            nc=nc,
        )
        return tuple(outs)

    devices = jax.devices()[:NCORES]
    mesh = Mesh(np.asarray(devices), ("core",))
    in_specs = (PartitionSpec("core"),) * (n_params + n_outs)
    out_specs = (PartitionSpec("core"),) * n_outs
    sharded = jax.jit(
        shard_map(_body, mesh=mesh, in_specs=in_specs, out_specs=out_specs, check_rep=False),
        donate_argnums=donate,
        keep_unused=True,
    )
    _CACHE["runner"] = (sharded, in_names, out_names, out_avals, zero_shapes, n_params)
    return _CACHE["runner"]


def make_core_inputs(hidden_states, Wq, Wk, Wv, Wo, q_norm_w, k_norm_w, position_ids):
    """Host-side sharding: per-core input dicts."""
    xt = np.ascontiguousarray(hidden_states.reshape(T, D).astype(np.float32).T)  # [D, T]
    inv_freq = 1.0 / (ROPE_THETA ** (np.arange(0, HD, 2, dtype=np.float32) / HD))
    ang = position_ids.astype(np.float32)[None, :] * inv_freq[:, None]  # [128, S]
    cos_t = np.cos(ang).astype(np.float32)
    sin_t = np.sin(ang).astype(np.float32)
    w1pq = np.ascontiguousarray((1.0 + q_norm_w.astype(np.float32)).reshape(2, 128).T)
    w1pk = np.ascontiguousarray((1.0 + k_norm_w.astype(np.float32)).reshape(2, 128).T)
    maps = []
    for c in range(NCORES):
        kvh = c // 2
        maps.append(
            {
                "x0": np.ascontiguousarray(xt[:, c * SH : (c + 1) * SH]),
                "x1": np.ascontiguousarray(xt[:, S + c * SH : S + (c + 1) * SH]),
                "wq": np.ascontiguousarray(Wq[:, c * EC : (c + 1) * EC].astype(np.float32)),
                "wk": np.ascontiguousarray(Wk[:, kvh * HD : (kvh + 1) * HD].astype(np.float32)),
                "wv": np.ascontiguousarray(Wv[:, kvh * HD : (kvh + 1) * HD].astype(np.float32)),
                "wo": np.ascontiguousarray(Wo[c * EC : (c + 1) * EC, :].astype(np.float32)),
                "cos_t": cos_t,
                "sin_t": sin_t,
                "w1pq": w1pq,
                "w1pk": w1pk,
            }
        )
    return maps


def kernel(hidden_states, Wq, Wk, Wv, Wo, q_norm_w, k_norm_w, position_ids):
    maps = make_core_inputs(
        np.asarray(hidden_states), np.asarray(Wq), np.asarray(Wk), np.asarray(Wv),
        np.asarray(Wo), np.asarray(q_norm_w), np.asarray(k_norm_w), np.asarray(position_ids),
    )
    sharded, in_names, out_names, out_avals, zero_shapes, n_params = _get_runner()
    concat_in = [
        np.concatenate([maps[c][nm] for c in range(NCORES)], axis=0) for nm in in_names
    ]
    concat_zeros = [
        np.zeros((NCORES * shp[0], *shp[1:]), dt) for shp, dt in zero_shapes
    ]
    out_arrs = sharded(*concat_in, *concat_zeros)
    yi = out_names.index("y")
    ys = np.asarray(out_arrs[yi]).reshape(NCORES, 2 * SH, D)
    out = np.empty((T, D), np.float32)
    for c in range(NCORES):
        out[c * SH : (c + 1) * SH] = ys[c, 0:SH]
        out[S + c * SH : S + (c + 1) * SH] = ys[c, SH : 2 * SH]
    return out.reshape(B, S, D)

